# revision 5
# baseline (speedup 1.0000x reference)
"""CoAtten2 Trainium2 kernel: 8-way tensor-parallel over one TRN2 chip.

Reference computation (C=1024, H=W=64, HW=4096):
    q   = (Wq @ Xm + bq)  viewed [1024, 2048] then transposed
    kf  = (Wk1 @ Xf + bk1) viewed [1024, 2048]
    kl  = (Wk2 @ Xl + bk2) viewed [1024, 2048]
    att = softmax(kf @ q) + softmax(kl @ q)          # [1024, 1024]
    out = gamma * (att @ (Wv @ Xm + bv)) + (Xf + Xl)/2

Sharding (per core d of 8):
  - spatial slice S_d = columns [512d, 512(d+1)) of the flattened [1024, 4096]
    inputs; projections computed transposed (CqT = Xm_d.T @ Wq.T + bq etc) so
    the contraction (channel) dim rides the SBUF partition axis.
  - CqT/CkfT/CklT exchanged with 8-core AllGathers; each core then reads its
    [2048, 128] K-operand slice with one dynamic-offset DMA (offsets arrive as
    per-core input data, keeping the SPMD program identical on all cores).
  - Channel indices are permuted (J' = 512t + o <-> j = 2o + t) to make the
    torch-style reshape contiguous; the permutation is folded into the host-side
    Wv/bv prep and the output DMA access pattern. gamma is folded into Wv/bv.
  - att rows are block-sharded; softmax is a free-dim reduction (exp in-place in
    PSUM); the summed attention is AllGathered in bf16 and each core computes
    its output column slice att @ V_d plus the residual.

SBUF slot tenancy (pool `pw`, one slot per tag):
  xf0..7  -> attT0..7 (bf16)         xm0..7 -> r0..7
  xl0..3  -> xfp stream              xl4..7 -> xlp stream
  wq0..3  -> cq stream               wq4/5/6 -> att_f/att_l/att_sum (bf16)
  wq7+wq4 -> out staging             wv0..7 -> v0..7 (bf16)
"""

import sys

sys.path.insert(0, "/opt/trn_rl_repo")

import numpy as np

import concourse.bass as bass
import concourse.bacc as bacc
import concourse.mybir as mybir
from concourse import tile
from concourse.bass_utils import run_bass_kernel_spmd

F32 = mybir.dt.float32
F32R = mybir.dt.float32r
BF16 = mybir.dt.bfloat16

C = 1024
HW = 4096
S = 512          # spatial columns per core
CH = 512         # C // 2 (projection output channels)
NCORES = 8

_CACHE: dict = {}


def _build():
    nc = bacc.Bacc("TRN2", target_bir_lowering=False, debug=False, num_devices=NCORES)

    # per-core external inputs
    xm = nc.declare_dram_parameter("xm", [C, S], F32, isOutput=False)
    xf = nc.declare_dram_parameter("xf", [C, S], F32, isOutput=False)
    xl = nc.declare_dram_parameter("xl", [C, S], F32, isOutput=False)
    wq = nc.declare_dram_parameter("wq", [C, CH], F32, isOutput=False)   # Wq.T
    wk1 = nc.declare_dram_parameter("wk1", [C, CH], F32, isOutput=False)
    wk2 = nc.declare_dram_parameter("wk2", [C, CH], F32, isOutput=False)
    wv = nc.declare_dram_parameter("wv", [C, C], F32, isOutput=False)    # (g*Wv)[permJ].T
    bqr = nc.declare_dram_parameter("bqr", [128, CH], F32, isOutput=False)
    bk1r = nc.declare_dram_parameter("bk1r", [128, CH], F32, isOutput=False)
    bk2r = nc.declare_dram_parameter("bk2r", [128, CH], F32, isOutput=False)
    bvp = nc.declare_dram_parameter("bvp", [128, 8], F32, isOutput=False)
    offs = nc.declare_dram_parameter("offs", [1, 4], mybir.dt.int32, isOutput=False)
    xfp = nc.declare_dram_parameter("xfp", [C, S], F32, isOutput=False)  # perm rows
    xlp = nc.declare_dram_parameter("xlp", [C, S], F32, isOutput=False)
    out_ext = nc.declare_dram_parameter("out", [C, S], F32, isOutput=True)

    # internal DRAM (collective bounce buffers)
    agq_in = nc.dram_tensor("agq_in", [S, CH], F32)
    agq_out = nc.dram_tensor("agq_out", [HW, CH], F32, addr_space="Shared")
    kf_in = nc.dram_tensor("kf_in", [S, CH], F32)
    kf_out = nc.dram_tensor("kf_out", [HW, CH], F32, addr_space="Shared")
    kl_in = nc.dram_tensor("kl_in", [S, CH], F32)
    kl_out = nc.dram_tensor("kl_out", [HW, CH], F32, addr_space="Shared")
    att_in = nc.dram_tensor("att_in", [128, C], BF16)
    att_out = nc.dram_tensor("att_out", [C, C], BF16, addr_space="Shared")

    groups8 = [list(range(NCORES))]

    with tile.TileContext(nc) as tc:
        with (
            tc.tile_pool(name="pw", bufs=1) as pw,
            tc.tile_pool(name="pst", bufs=3) as pst,
            tc.tile_pool(name="psc", bufs=2) as psc,
            tc.tile_pool(name="pps", bufs=2, space="PSUM") as pps,
            tc.tile_pool(name="plog", bufs=1, space="PSUM") as plog,
        ):
            # ---- input loads -------------------------------------------------
            def load8(dram, width, tag, dt=F32R):
                ts = []
                for c in range(8):
                    t = pw.tile([128, width], dt, tag=f"{tag}{c}")
                    nc.sync.dma_start(t[:], dram[128 * c:128 * (c + 1), :].bitcast(dt))
                    ts.append(t)
                return ts

            xf_t = load8(xf, S, "xf")
            wk1_t = load8(wk1, CH, "wk1")
            xm_t = load8(xm, S, "xm")
            wq_t = load8(wq, CH, "wq")
            xl_t = load8(xl, S, "xl")
            wk2_t = load8(wk2, CH, "wk2")

            bk1_t = pw.tile([128, CH], F32, tag="bk1")
            nc.sync.dma_start(bk1_t[:], bk1r[:, :])
            bq_t = pw.tile([128, CH], F32, tag="bq")
            nc.sync.dma_start(bq_t[:], bqr[:, :])
            bk2_t = pw.tile([128, CH], F32, tag="bk2")
            nc.sync.dma_start(bk2_t[:], bk2r[:, :])
            bv_t = pw.tile([128, 8], F32, tag="bv")
            nc.sync.dma_start(bv_t[:], bvp[:, :])
            offs_t = pw.tile([1, 4], mybir.dt.int32, tag="offs")
            nc.sync.dma_start(offs_t[:], offs[:, :])

            # ---- phase 1: projections (transposed, spatially sharded) --------
            def proj_ckt(x_tiles, w_tiles, bias_t, ag_in):
                # CkT_d[s, o] = sum_c X[c, s] * WT[c, o] + b[o]   -> [512, 512]
                for ssub in range(4):
                    ps = pps.tile([128, CH], F32, tag="mm")
                    for c in range(8):
                        nc.tensor.matmul(
                            ps[:],
                            x_tiles[c][:, 128 * ssub:128 * (ssub + 1)],
                            w_tiles[c][:],
                            start=(c == 0),
                            stop=(c == 7),
                        )
                    st = pst.tile([128, CH], F32, tag="stage")
                    nc.vector.tensor_add(st[:], ps[:], bias_t[:])
                    nc.sync.dma_start(ag_in[128 * ssub:128 * (ssub + 1), :], st[:])

            proj_ckt(xf_t, wk1_t, bk1_t, kf_in)
            nc.gpsimd.collective_compute(
                "AllGather",
                mybir.AluOpType.bypass,
                ins=[kf_in[:]],
                outs=[kf_out[:]],
                replica_groups=groups8,
            )

            proj_ckt(xm_t, wq_t, bq_t, agq_in)
            nc.gpsimd.collective_compute(
                "AllGather",
                mybir.AluOpType.bypass,
                ins=[agq_in[:]],
                outs=[agq_out[:]],
                replica_groups=groups8,
            )

            proj_ckt(xl_t, wk2_t, bk2_t, kl_in)
            nc.gpsimd.collective_compute(
                "AllGather",
                mybir.AluOpType.bypass,
                ins=[kl_in[:]],
                outs=[kl_out[:]],
                replica_groups=groups8,
            )

            # ---- V projection (local): V[J', hw_d] in bf16, bias per J' ------
            wv_t = load8(wv, C, "wv")
            v_sb = []
            for j in range(8):
                ps = pps.tile([128, S], F32, tag="mm")
                for c in range(8):
                    nc.tensor.matmul(
                        ps[:],
                        wv_t[c][:, 128 * j:128 * (j + 1)],
                        xm_t[c][:],
                        start=(c == 0),
                        stop=(c == 7),
                    )
                v = pw.tile([128, S], BF16, tag=f"v{j}")
                nc.vector.tensor_scalar_add(v[:], ps[:], bv_t[:, j:j + 1])
                v_sb.append(v)

            # ---- residual: R[e] = 0.5 * (xf + xl) on permuted rows -----------
            r_sb = []
            for e in range(8):
                a = pw.tile([128, S], F32, tag=f"xl{e % 4}")
                nc.sync.dma_start(a[:], xfp[128 * e:128 * (e + 1), :])
                b = pw.tile([128, S], F32, tag=f"xl{4 + e % 4}")
                nc.sync.dma_start(b[:], xlp[128 * e:128 * (e + 1), :])
                r = pw.tile([128, S], F32, tag=f"xm{e}")
                nc.vector.tensor_add(r[:], a[:], b[:])
                nc.scalar.mul(r[:], r[:], 0.5)
                r_sb.append(r)

            # ---- phase 2: logits + softmax ----------------------------------
            # per-core operand slice [2048, 128] of the gathered CkT stacks,
            # addressed by host-provided offsets (SPMD: one shared program).
            rn = nc.gpsimd.alloc_register("rn_slice")
            nc.gpsimd.reg_load(rn, offs_t[0:1, 0:1])
            vn = nc.gpsimd.snap(rn, donate=True, min_val=0, max_val=16)
            rcr = nc.gpsimd.alloc_register("rc_slice")
            nc.gpsimd.reg_load(rcr, offs_t[0:1, 1:2])
            vc = nc.gpsimd.snap(rcr, donate=True, min_val=0, max_val=384)

            ckf_loc = pw.tile([128, 16, 128], F32R, tag="ckf")
            kf_view = kf_out[:].rearrange("(n p) m -> p n m", p=128).bitcast(F32R)
            nc.gpsimd.dma_start(
                ckf_loc[:], kf_view[:, bass.ds(vn, 16), bass.ds(vc, 128)]
            )
            ckl_loc = pw.tile([128, 16, 128], F32R, tag="ckl")
            kl_view = kl_out[:].rearrange("(n p) m -> p n m", p=128).bitcast(F32R)
            nc.gpsimd.dma_start(
                ckl_loc[:], kl_view[:, bass.ds(vn, 16), bass.ds(vc, 128)]
            )

            lg_f = plog.tile([128, C], F32, tag="lf")
            lg_l = plog.tile([128, C], F32, tag="ll")
            for tp in range(2):
                for k in range(16):
                    cq = pw.tile([128, CH], F32R, tag=f"wq{k % 4}")
                    row = 2048 * tp + 128 * k
                    nc.sync.dma_start(
                        cq[:], agq_out[row:row + 128, :].bitcast(F32R)
                    )
                    nc.tensor.matmul(
                        lg_f[:, CH * tp:CH * (tp + 1)],
                        ckf_loc[:, k, :],
                        cq[:],
                        start=(k == 0),
                        stop=(k == 15),
                    )
                    nc.tensor.matmul(
                        lg_l[:, CH * tp:CH * (tp + 1)],
                        ckl_loc[:, k, :],
                        cq[:],
                        start=(k == 0),
                        stop=(k == 15),
                    )

            att_parts = []
            for slot, lg in (("wq4", lg_f), ("wq5", lg_l)):
                mxn = psc.tile([128, 1], F32, tag="mx")
                nc.vector.reduce_max(
                    mxn[:], lg[:], axis=mybir.AxisListType.X, negate=True
                )
                sm = psc.tile([128, 1], F32, tag="sm")
                nc.scalar.activation(
                    lg[:],
                    lg[:],
                    mybir.ActivationFunctionType.Exp,
                    bias=mxn[:, 0:1],
                    accum_out=sm[:, 0:1],
                )
                rcp = psc.tile([128, 1], F32, tag="rc")
                nc.vector.reciprocal(rcp[:], sm[:])
                at = pw.tile([128, C], BF16, tag=slot)
                nc.vector.tensor_scalar_mul(at[:], lg[:], rcp[:, 0:1])
                att_parts.append(at)
            att_sum = pw.tile([128, C], BF16, tag="wq6")
            nc.vector.tensor_add(att_sum[:], att_parts[0][:], att_parts[1][:])
            nc.sync.dma_start(att_in[:], att_sum[:])
            nc.gpsimd.collective_compute(
                "AllGather",
                mybir.AluOpType.bypass,
                ins=[att_in[:]],
                outs=[att_out[:]],
                replica_groups=groups8,
            )

            # ---- phase 3: out[:, hw_d] = att @ V_d + R ----------------------
            att_t = []
            for k in range(8):
                t = pw.tile([128, C], BF16, tag=f"xf{k}")
                nc.sync.dma_start(
                    t[:], att_out[:, 128 * k:128 * (k + 1)], transpose=True
                )
                att_t.append(t)
            out_v = out_ext[:].rearrange("(o t) w -> t o w", t=2)
            for e in range(8):
                ps = pps.tile([128, S], F32, tag="mm")
                for k in range(8):
                    nc.tensor.matmul(
                        ps[:],
                        att_t[k][:, 128 * e:128 * (e + 1)],
                        v_sb[k][:],
                        start=(k == 0),
                        stop=(k == 7),
                    )
                ost = pw.tile([128, S], F32, tag=f"wq{7 if e % 2 else 4}")
                nc.vector.tensor_add(ost[:], ps[:], r_sb[e][:])
                nc.sync.dma_start(
                    out_v[e // 4, 128 * (e % 4):128 * (e % 4 + 1), :], ost[:]
                )

    nc.compile()
    return nc


def _prep_inputs(x_f, x_m, x_l, Wq, bq, Wk1, bk1, Wk2, bk2, Wv, bv, gamma):
    Xf = np.ascontiguousarray(x_f.reshape(C, HW), dtype=np.float32)
    Xm = np.ascontiguousarray(x_m.reshape(C, HW), dtype=np.float32)
    Xl = np.ascontiguousarray(x_l.reshape(C, HW), dtype=np.float32)
    g = np.float32(np.asarray(gamma).reshape(-1)[0])

    permJ = 2 * (np.arange(C) % 512) + np.arange(C) // 512  # J' -> global j
    wv_full = np.ascontiguousarray((g * Wv)[permJ, :].T, dtype=np.float32)
    bv_perm = (g * bv)[permJ].astype(np.float32)

    wq_full = np.ascontiguousarray(Wq.T, dtype=np.float32)
    wk1_full = np.ascontiguousarray(Wk1.T, dtype=np.float32)
    wk2_full = np.ascontiguousarray(Wk2.T, dtype=np.float32)
    bqr = np.ascontiguousarray(np.broadcast_to(bq, (128, CH)), dtype=np.float32)
    bk1r = np.ascontiguousarray(np.broadcast_to(bk1, (128, CH)), dtype=np.float32)
    bk2r = np.ascontiguousarray(np.broadcast_to(bk2, (128, CH)), dtype=np.float32)
    bvp = np.ascontiguousarray(bv_perm.reshape(8, 128).T)
    Xfp = Xf[permJ, :]
    Xlp = Xl[permJ, :]

    in_maps = []
    for d in range(NCORES):
        sl = slice(S * d, S * (d + 1))
        in_maps.append({
            "offs": np.array([[16 * (d // 4), 128 * (d % 4), 0, 0]], dtype=np.int32),
            "xm": np.ascontiguousarray(Xm[:, sl]),
            "xf": np.ascontiguousarray(Xf[:, sl]),
            "xl": np.ascontiguousarray(Xl[:, sl]),
            "wq": wq_full,
            "wk1": wk1_full,
            "wk2": wk2_full,
            "wv": wv_full,
            "bqr": bqr,
            "bk1r": bk1r,
            "bk2r": bk2r,
            "bvp": bvp,
            "xfp": np.ascontiguousarray(Xfp[:, sl]),
            "xlp": np.ascontiguousarray(Xlp[:, sl]),
        })
    return in_maps


def _run(inputs: dict, trace: bool = False, **kw):
    if "nc" not in _CACHE:
        _CACHE["nc"] = _build()
    nc = _CACHE["nc"]
    in_maps = _prep_inputs(**inputs)
    res = run_bass_kernel_spmd(nc, in_maps, list(range(NCORES)), trace=trace, **kw)
    out = np.empty((C, HW), np.float32)
    for d in range(NCORES):
        out[:, S * d:S * (d + 1)] = res.results[d]["out"]
    return out.reshape(1, C, 64, 64), res


def kernel(**inputs) -> np.ndarray:
    out, _ = _run(inputs)
    return out


# revision 7
# speedup vs baseline: 1.1278x; 1.1278x over previous
"""CoAtten2 Trainium2 kernel: 8-way tensor-parallel over one TRN2 chip.

Reference computation (C=1024, H=W=64, HW=4096):
    q   = (Wq @ Xm + bq)  viewed [1024, 2048] then transposed
    kf  = (Wk1 @ Xf + bk1) viewed [1024, 2048]
    kl  = (Wk2 @ Xl + bk2) viewed [1024, 2048]
    att = softmax(kf @ q) + softmax(kl @ q)          # [1024, 1024]
    out = gamma * (att @ (Wv @ Xm + bv)) + (Xf + Xl)/2

Sharding (per core d of 8):
  - spatial slice S_d = columns [512d, 512(d+1)) of the flattened [1024, 4096]
    inputs; projections computed transposed (CqT = Xm_d.T @ Wq.T + bq etc) so
    the contraction (channel) dim rides the SBUF partition axis.
  - CqT/CkfT/CklT exchanged with 8-core AllGathers; each core then reads its
    [2048, 128] K-operand slice with one dynamic-offset DMA (offsets arrive as
    per-core input data, keeping the SPMD program identical on all cores).
  - Channel indices are permuted (J' = 512t + o <-> j = 2o + t) to make the
    torch-style reshape contiguous; the permutation is folded into the host-side
    Wv/bv prep and the output DMA access pattern. gamma is folded into Wv/bv.
  - att rows are block-sharded; softmax is a free-dim reduction (exp in-place in
    PSUM); the summed attention is AllGathered in bf16 and each core computes
    its output column slice att @ V_d plus the residual.

SBUF slot tenancy (pool `pw`, one slot per tag):
  xf0..7  -> attT0..7 (bf16)         xm0..7 -> r0..7
  xl0..3  -> xfp stream              xl4..7 -> xlp stream
  wq0..3  -> cq stream               wq4/5/6 -> att_f/att_l/att_sum (bf16)
  wq7+wq4 -> out staging             wv0..7 -> v0..7 (bf16)
"""

import sys

sys.path.insert(0, "/opt/trn_rl_repo")

import numpy as np

import concourse.bass as bass
import concourse.bacc as bacc
import concourse.mybir as mybir
from concourse import tile
from concourse.bass_utils import run_bass_kernel_spmd

F32 = mybir.dt.float32
F32R = mybir.dt.float32r
BF16 = mybir.dt.bfloat16

C = 1024
HW = 4096
S = 512          # spatial columns per core
CH = 512         # C // 2 (projection output channels)
NCORES = 8

_CACHE: dict = {}


def _build():
    nc = bacc.Bacc("TRN2", target_bir_lowering=False, debug=False, num_devices=NCORES)

    # per-core external inputs
    xm = nc.declare_dram_parameter("xm", [C, S], F32, isOutput=False)
    xf = nc.declare_dram_parameter("xf", [C, S], F32, isOutput=False)
    xl = nc.declare_dram_parameter("xl", [C, S], F32, isOutput=False)
    wq = nc.declare_dram_parameter("wq", [C, CH], F32, isOutput=False)   # Wq.T
    wk1 = nc.declare_dram_parameter("wk1", [C, CH], F32, isOutput=False)
    wk2 = nc.declare_dram_parameter("wk2", [C, CH], F32, isOutput=False)
    wv = nc.declare_dram_parameter("wv", [C, C], F32, isOutput=False)    # (g*Wv)[permJ].T
    bqr = nc.declare_dram_parameter("bqr", [128, CH], F32, isOutput=False)
    bk1r = nc.declare_dram_parameter("bk1r", [128, CH], F32, isOutput=False)
    bk2r = nc.declare_dram_parameter("bk2r", [128, CH], F32, isOutput=False)
    bvp = nc.declare_dram_parameter("bvp", [128, 8], F32, isOutput=False)
    offs = nc.declare_dram_parameter("offs", [1, 4], mybir.dt.int32, isOutput=False)
    xfp = nc.declare_dram_parameter("xfp", [C, S], F32, isOutput=False)  # perm rows
    xlp = nc.declare_dram_parameter("xlp", [C, S], F32, isOutput=False)
    out_ext = nc.declare_dram_parameter("out", [C, S], F32, isOutput=True)

    # internal DRAM (collective bounce buffers)
    agq_in = nc.dram_tensor("agq_in", [S, CH], F32)
    agq_out = nc.dram_tensor("agq_out", [HW, CH], F32, addr_space="Shared")
    # fused kf|kl AllToAll: sender d's slot e = [512 rows, kf cols O_e | kl cols O_e]
    kk_in = nc.dram_tensor("kk_in", [HW, 256], F32)
    kk_out = nc.dram_tensor("kk_out", [HW, 256], F32)
    dummy_in = nc.dram_tensor("dummy_in", [1, 128], F32)
    dummy_out = nc.dram_tensor("dummy_out", [8, 128], F32, addr_space="Shared")
    att_in = nc.dram_tensor("att_in", [128, C], BF16)
    att_out = nc.dram_tensor("att_out", [C, C], BF16, addr_space="Shared")

    groups8 = [list(range(NCORES))]

    with tile.TileContext(nc) as tc:
        with (
            tc.tile_pool(name="pw", bufs=1) as pw,
            tc.tile_pool(name="pst", bufs=3) as pst,
            tc.tile_pool(name="psc", bufs=2) as psc,
            tc.tile_pool(name="pps", bufs=2, space="PSUM") as pps,
            tc.tile_pool(name="plog", bufs=1, space="PSUM") as plog,
        ):
            # Fire a tiny collective immediately: the first collective of the
            # NEFF pays a cross-core alignment barrier (~50us incl. launch
            # skew); this absorbs it while input DMA + projections run.
            nc.gpsimd.collective_compute(
                "AllGather",
                mybir.AluOpType.bypass,
                ins=[dummy_in[:]],
                outs=[dummy_out[:]],
                replica_groups=groups8,
            )

            # ---- input loads -------------------------------------------------
            def load8(dram, width, tag, dt=F32R):
                ts = []
                for c in range(8):
                    t = pw.tile([128, width], dt, tag=f"{tag}{c}")
                    nc.sync.dma_start(t[:], dram[128 * c:128 * (c + 1), :].bitcast(dt))
                    ts.append(t)
                return ts

            xf_t = load8(xf, S, "xf")
            wk1_t = load8(wk1, CH, "wk1")
            xl_t = load8(xl, S, "xl")
            wk2_t = load8(wk2, CH, "wk2")
            xm_t = load8(xm, S, "xm")
            wq_t = load8(wq, CH, "wq")

            bk1_t = pw.tile([128, CH], F32, tag="bk1")
            nc.sync.dma_start(bk1_t[:], bk1r[:, :])
            bq_t = pw.tile([128, CH], F32, tag="bq")
            nc.sync.dma_start(bq_t[:], bqr[:, :])
            bk2_t = pw.tile([128, CH], F32, tag="bk2")
            nc.sync.dma_start(bk2_t[:], bk2r[:, :])
            bv_t = pw.tile([128, 8], F32, tag="bv")
            nc.sync.dma_start(bv_t[:], bvp[:, :])
            offs_t = pw.tile([1, 4], mybir.dt.int32, tag="offs")
            nc.sync.dma_start(offs_t[:], offs[:, :])

            # ---- phase 1: projections (transposed, spatially sharded) --------
            def proj_ckt(x_tiles, w_tiles, bias_t, a2a_col=None):
                # CkT_d[s, o] = sum_c X[c, s] * WT[c, o] + b[o]   -> [512, 512]
                for ssub in range(4):
                    ps = pps.tile([128, CH], F32, tag="mm")
                    for c in range(8):
                        nc.tensor.matmul(
                            ps[:],
                            x_tiles[c][:, 128 * ssub:128 * (ssub + 1)],
                            w_tiles[c][:],
                            start=(c == 0),
                            stop=(c == 7),
                        )
                    st = pst.tile([128, CH], F32, tag="stage")
                    nc.vector.tensor_add(st[:], ps[:], bias_t[:])
                    if a2a_col is None:
                        nc.sync.dma_start(
                            agq_in[128 * ssub:128 * (ssub + 1), :], st[:]
                        )
                    else:
                        # receiver-slot-major: slot e rows [512e+128ssub+p],
                        # cols [a2a_col, a2a_col+128) = this tensor's O_e block.
                        # O_e = 128*(e%4): receivers e and e+4 want the same
                        # column block, so write it to both slot halves.
                        dst = kk_in[:].rearrange("(g j s) m -> s g j m", g=2, j=4)
                        src = st[:].rearrange("p (j m) -> p j m", j=4)
                        for g in range(2):
                            nc.sync.dma_start(
                                dst[128 * ssub:128 * (ssub + 1), g, :,
                                    a2a_col:a2a_col + 128],
                                src,
                            )

            proj_ckt(xf_t, wk1_t, bk1_t, a2a_col=0)
            proj_ckt(xl_t, wk2_t, bk2_t, a2a_col=128)
            nc.gpsimd.collective_compute(
                "AllToAll",
                mybir.AluOpType.bypass,
                ins=[kk_in[:]],
                outs=[kk_out[:]],
                replica_groups=groups8,
            )

            proj_ckt(xm_t, wq_t, bq_t)
            nc.gpsimd.collective_compute(
                "AllGather",
                mybir.AluOpType.bypass,
                ins=[agq_in[:]],
                outs=[agq_out[:]],
                replica_groups=groups8,
            )

            # ---- V projection (local): V[J', hw_d] in bf16, bias per J' ------
            wv_t = load8(wv, C, "wv")
            v_sb = []
            for j in range(8):
                ps = pps.tile([128, S], F32, tag="mm")
                for c in range(8):
                    nc.tensor.matmul(
                        ps[:],
                        wv_t[c][:, 128 * j:128 * (j + 1)],
                        xm_t[c][:],
                        start=(c == 0),
                        stop=(c == 7),
                    )
                v = pw.tile([128, S], BF16, tag=f"v{j}")
                nc.vector.tensor_scalar_add(v[:], ps[:], bv_t[:, j:j + 1])
                v_sb.append(v)

            # ---- residual: R[e] = 0.5 * (xf + xl) on permuted rows -----------
            r_sb = []
            for e in range(8):
                a = pw.tile([128, S], F32, tag=f"xl{e % 4}")
                nc.sync.dma_start(a[:], xfp[128 * e:128 * (e + 1), :])
                b = pw.tile([128, S], F32, tag=f"xl{4 + e % 4}")
                nc.sync.dma_start(b[:], xlp[128 * e:128 * (e + 1), :])
                r = pw.tile([128, S], F32, tag=f"xm{e}")
                nc.vector.tensor_add(r[:], a[:], b[:])
                nc.scalar.mul(r[:], r[:], 0.5)
                r_sb.append(r)

            # ---- phase 2: logits + softmax ----------------------------------
            # per-core operand slice [2048, 128] of the gathered CkT stacks,
            # addressed by host-provided offsets (SPMD: one shared program).
            rn = nc.gpsimd.alloc_register("rn_slice")
            nc.gpsimd.reg_load(rn, offs_t[0:1, 0:1])
            vn = nc.gpsimd.snap(rn, donate=True, min_val=0, max_val=16)
            kk_view = kk_out[:].rearrange("(n p) m -> p n m", p=128).bitcast(F32R)
            ckf_loc = pw.tile([128, 16, 128], F32R, tag="ckf")
            nc.gpsimd.dma_start(
                ckf_loc[:], kk_view[:, bass.ds(vn, 16), 0:128]
            )
            ckl_loc = pw.tile([128, 16, 128], F32R, tag="ckl")
            nc.gpsimd.dma_start(
                ckl_loc[:], kk_view[:, bass.ds(vn, 16), 128:256]
            )

            lg_f = plog.tile([128, C], F32, tag="lf")
            lg_l = plog.tile([128, C], F32, tag="ll")
            for tp in range(2):
                for k in range(16):
                    cq = pw.tile([128, CH], F32R, tag=f"wq{k % 4}")
                    row = 2048 * tp + 128 * k
                    nc.sync.dma_start(
                        cq[:], agq_out[row:row + 128, :].bitcast(F32R)
                    )
                    nc.tensor.matmul(
                        lg_f[:, CH * tp:CH * (tp + 1)],
                        ckf_loc[:, k, :],
                        cq[:],
                        start=(k == 0),
                        stop=(k == 15),
                    )
                    nc.tensor.matmul(
                        lg_l[:, CH * tp:CH * (tp + 1)],
                        ckl_loc[:, k, :],
                        cq[:],
                        start=(k == 0),
                        stop=(k == 15),
                    )

            att_parts = []
            for slot, lg in (("wq4", lg_f), ("wq5", lg_l)):
                mxn = psc.tile([128, 1], F32, tag="mx")
                nc.vector.reduce_max(
                    mxn[:], lg[:], axis=mybir.AxisListType.X, negate=True
                )
                sm = psc.tile([128, 1], F32, tag="sm")
                nc.scalar.activation(
                    lg[:],
                    lg[:],
                    mybir.ActivationFunctionType.Exp,
                    bias=mxn[:, 0:1],
                    accum_out=sm[:, 0:1],
                )
                rcp = psc.tile([128, 1], F32, tag="rc")
                nc.vector.reciprocal(rcp[:], sm[:])
                at = pw.tile([128, C], BF16, tag=slot)
                nc.vector.tensor_scalar_mul(at[:], lg[:], rcp[:, 0:1])
                att_parts.append(at)
            att_sum = pw.tile([128, C], BF16, tag="wq6")
            nc.vector.tensor_add(att_sum[:], att_parts[0][:], att_parts[1][:])
            nc.sync.dma_start(att_in[:], att_sum[:])
            nc.gpsimd.collective_compute(
                "AllGather",
                mybir.AluOpType.bypass,
                ins=[att_in[:]],
                outs=[att_out[:]],
                replica_groups=groups8,
            )

            # ---- phase 3: out[:, hw_d] = att @ V_d + R ----------------------
            att_t = []
            for k in range(8):
                t = pw.tile([128, C], BF16, tag=f"xf{k}")
                nc.sync.dma_start(
                    t[:], att_out[:, 128 * k:128 * (k + 1)], transpose=True
                )
                att_t.append(t)
            out_v = out_ext[:].rearrange("(o t) w -> t o w", t=2)
            for e in range(8):
                ps = pps.tile([128, S], F32, tag="mm")
                for k in range(8):
                    nc.tensor.matmul(
                        ps[:],
                        att_t[k][:, 128 * e:128 * (e + 1)],
                        v_sb[k][:],
                        start=(k == 0),
                        stop=(k == 7),
                    )
                ost = pw.tile([128, S], F32, tag=f"wq{7 if e % 2 else 4}")
                nc.vector.tensor_add(ost[:], ps[:], r_sb[e][:])
                nc.sync.dma_start(
                    out_v[e // 4, 128 * (e % 4):128 * (e % 4 + 1), :], ost[:]
                )

    nc.compile()
    return nc


def _prep_inputs(x_f, x_m, x_l, Wq, bq, Wk1, bk1, Wk2, bk2, Wv, bv, gamma):
    Xf = np.ascontiguousarray(x_f.reshape(C, HW), dtype=np.float32)
    Xm = np.ascontiguousarray(x_m.reshape(C, HW), dtype=np.float32)
    Xl = np.ascontiguousarray(x_l.reshape(C, HW), dtype=np.float32)
    g = np.float32(np.asarray(gamma).reshape(-1)[0])

    permJ = 2 * (np.arange(C) % 512) + np.arange(C) // 512  # J' -> global j
    wv_full = np.ascontiguousarray((g * Wv)[permJ, :].T, dtype=np.float32)
    bv_perm = (g * bv)[permJ].astype(np.float32)

    wq_full = np.ascontiguousarray(Wq.T, dtype=np.float32)
    wk1_full = np.ascontiguousarray(Wk1.T, dtype=np.float32)
    wk2_full = np.ascontiguousarray(Wk2.T, dtype=np.float32)
    bqr = np.ascontiguousarray(np.broadcast_to(bq, (128, CH)), dtype=np.float32)
    bk1r = np.ascontiguousarray(np.broadcast_to(bk1, (128, CH)), dtype=np.float32)
    bk2r = np.ascontiguousarray(np.broadcast_to(bk2, (128, CH)), dtype=np.float32)
    bvp = np.ascontiguousarray(bv_perm.reshape(8, 128).T)
    Xfp = Xf[permJ, :]
    Xlp = Xl[permJ, :]

    in_maps = []
    for d in range(NCORES):
        sl = slice(S * d, S * (d + 1))
        in_maps.append({
            "offs": np.array([[16 * (d // 4), 128 * (d % 4), 0, 0]], dtype=np.int32),
            "xm": np.ascontiguousarray(Xm[:, sl]),
            "xf": np.ascontiguousarray(Xf[:, sl]),
            "xl": np.ascontiguousarray(Xl[:, sl]),
            "wq": wq_full,
            "wk1": wk1_full,
            "wk2": wk2_full,
            "wv": wv_full,
            "bqr": bqr,
            "bk1r": bk1r,
            "bk2r": bk2r,
            "bvp": bvp,
            "xfp": np.ascontiguousarray(Xfp[:, sl]),
            "xlp": np.ascontiguousarray(Xlp[:, sl]),
        })
    return in_maps


def _run(inputs: dict, trace: bool = False, **kw):
    if "nc" not in _CACHE:
        _CACHE["nc"] = _build()
    nc = _CACHE["nc"]
    in_maps = _prep_inputs(**inputs)
    res = run_bass_kernel_spmd(nc, in_maps, list(range(NCORES)), trace=trace, **kw)
    out = np.empty((C, HW), np.float32)
    for d in range(NCORES):
        out[:, S * d:S * (d + 1)] = res.results[d]["out"]
    return out.reshape(1, C, 64, 64), res


def kernel(**inputs) -> np.ndarray:
    out, _ = _run(inputs)
    return out
